# revision 4
# baseline (speedup 1.0000x reference)
"""ColumnParallelLinearWithMoE Trainium2 kernel.

Expert-parallel: expert e -> NeuronCore e. Each core computes
    y_e [8192, 512] = x_e [8192, 1024] @ W_e.T [1024, 512] + b_e
where x_e = input_[idx_list[e]] flattened over (per, seq).

Routing gather/scatter and the x transpose (to put the contraction dim on
SBUF partitions) happen on the host; the device does the dense matmul.
"""

import sys

if "/opt/trn_rl_repo" not in sys.path:
    sys.path.insert(0, "/opt/trn_rl_repo")

import numpy as np

# Problem constants (hardcoded per harness contract).
E = 8
BS = 64
S = 1024
D = 1024
OPP = 512
P = 128
TOK = (BS // E) * S  # 8192 tokens per expert
KT = D // P          # 8 contraction tiles
TW = 512             # token-superblock width staged in SBUF
NSUP = TOK // TW
TPS = TW // P        # token tiles (of 128) per superblock

# Matmul dtype variant: "f32" (exact, slow), "f32r" (fast fp32 path),
# "bf16" (host-cast, fastest DMA).
VARIANT = "f32"

_programs: dict[str, tuple] = {}


def _build(variant: str):
    import concourse.bacc as bacc
    import concourse.tile as tile
    from concourse import mybir

    if variant == "f32":
        mm_dt = mybir.dt.float32
        np_in = np.float32
    elif variant == "f32r":
        mm_dt = mybir.dt.float32r
        np_in = np.float32
    elif variant == "bf16":
        import ml_dtypes

        mm_dt = mybir.dt.bfloat16
        np_in = ml_dtypes.bfloat16
    else:
        raise ValueError(variant)

    nc = bacc.Bacc(None, target_bir_lowering=False, debug=False)

    xt = nc.dram_tensor("xt", [D, TOK], mm_dt, kind="ExternalInput")
    wt = nc.dram_tensor("wt", [D, OPP], mm_dt, kind="ExternalInput")
    bias = nc.dram_tensor("bias", [P, OPP], mybir.dt.float32, kind="ExternalInput")
    y = nc.dram_tensor("y", [TOK, OPP], mybir.dt.float32, kind="ExternalOutput")

    # Batched-DMA views: one dma_start per x/y superblock (split across all
    # 16 SDMA engines), so the issuing engine isn't the bottleneck.
    xt_r = xt.rearrange("(k p) t -> p k t", p=P)        # [128, KT, TOK]
    y_r = y.rearrange("(s j p) c -> p s j c", p=P, j=TPS)  # [128, NSUP, TPS, OPP]

    with tile.TileContext(nc) as tc:
        with (
            tc.tile_pool(name="wpool", bufs=1) as wpool,
            tc.tile_pool(name="bpool", bufs=1) as bpool,
            tc.tile_pool(name="xpool", bufs=3) as xpool,
            tc.tile_pool(name="opool", bufs=2) as opool,
            tc.tile_pool(name="pspool", bufs=4, space="PSUM") as pspool,
        ):
            bias_sb = bpool.tile([P, OPP], mybir.dt.float32)
            nc.sync.dma_start(out=bias_sb[:], in_=bias[:])

            w_sb = []
            for k in range(KT):
                wtile = wpool.tile([P, OPP], mm_dt, tag=f"w{k}")
                nc.sync.dma_start(out=wtile[:], in_=wt[k * P : (k + 1) * P, :])
                w_sb.append(wtile)

            for s in range(NSUP):
                x_sb = xpool.tile([P, KT, TW], mm_dt, tag="x")
                nc.sync.dma_start(
                    out=x_sb[:], in_=xt_r[:, :, s * TW : (s + 1) * TW]
                )
                o_sb = opool.tile([P, TPS, OPP], mybir.dt.float32, tag="o")
                for j in range(TPS):
                    ps = pspool.tile([P, OPP], mybir.dt.float32)
                    for k in range(KT):
                        nc.tensor.matmul(
                            ps[:],
                            x_sb[:, k, j * P : (j + 1) * P],
                            w_sb[k][:],
                            start=(k == 0),
                            stop=(k == KT - 1),
                        )
                    nc.vector.tensor_add(o_sb[:, j, :], ps[:], bias_sb[:])
                nc.sync.dma_start(out=y_r[:, s, :, :], in_=o_sb[:])

    nc.compile()
    return nc, np_in


def _get_program(variant: str):
    if variant not in _programs:
        _programs[variant] = _build(variant)
    return _programs[variant]


def kernel(input_, idx_list, W, b, **_ignored):
    from concourse.bass_utils import run_bass_kernel_spmd

    input_ = np.asarray(input_)
    idx = np.asarray(idx_list).astype(np.int64)
    W = np.asarray(W, dtype=np.float32)
    b = np.asarray(b, dtype=np.float32)

    nc, np_in = _get_program(VARIANT)

    in_maps = []
    for e in range(E):
        xg = input_[idx[e]].reshape(TOK, D).astype(np.float32, copy=False)
        xtr = np.ascontiguousarray(xg.T).astype(np_in)
        wtr = np.ascontiguousarray(W[e].T).astype(np_in)
        bias_bc = np.ascontiguousarray(
            np.broadcast_to(b[e][None, :], (P, OPP))
        ).astype(np.float32)
        in_maps.append({"xt": xtr, "wt": wtr, "bias": bias_bc})

    res = run_bass_kernel_spmd(nc, in_maps, core_ids=list(range(E)))

    out = np.zeros((BS, S, E * OPP), dtype=input_.dtype)
    for e in range(E):
        ye = np.asarray(res.results[e]["y"], dtype=input_.dtype)
        out[idx[e], :, e * OPP : (e + 1) * OPP] = ye.reshape(BS // E, S, OPP)
    return out


# revision 6
# speedup vs baseline: 1.1148x; 1.1148x over previous
"""ColumnParallelLinearWithMoE Trainium2 kernel.

Expert-parallel: expert e -> NeuronCore e. Each core computes
    y_e [8192, 512] = x_e [8192, 1024] @ W_e.T [1024, 512] + b_e
where x_e = input_[idx_list[e]] flattened over (per, seq).

Routing gather/scatter and the x transpose (to put the contraction dim on
SBUF partitions) happen on the host; the device does the dense matmul.
"""

import sys

if "/opt/trn_rl_repo" not in sys.path:
    sys.path.insert(0, "/opt/trn_rl_repo")

import numpy as np

# Problem constants (hardcoded per harness contract).
E = 8
BS = 64
S = 1024
D = 1024
OPP = 512
P = 128
TOK = (BS // E) * S  # 8192 tokens per expert
KT = D // P          # 8 contraction tiles
TW = 512             # token-superblock width staged in SBUF
NSUP = TOK // TW
TPS = TW // P        # token tiles (of 128) per superblock

# Matmul dtype variant: "f32" (exact, slow), "f32r" (fast fp32 path),
# "bf16" (host-cast, fastest DMA).
VARIANT = "f32"

_programs: dict[str, tuple] = {}


def _build(variant: str):
    import concourse.bacc as bacc
    import concourse.tile as tile
    from concourse import mybir

    if variant == "f32":
        mm_dt = mybir.dt.float32
        np_in = np.float32
    elif variant == "f32r":
        mm_dt = mybir.dt.float32r
        np_in = np.float32
    elif variant == "bf16":
        import ml_dtypes

        mm_dt = mybir.dt.bfloat16
        np_in = ml_dtypes.bfloat16
    else:
        raise ValueError(variant)

    nc = bacc.Bacc(None, target_bir_lowering=False, debug=False)

    xt = nc.dram_tensor("xt", [D, TOK], mm_dt, kind="ExternalInput")
    wt = nc.dram_tensor("wt", [D, OPP], mm_dt, kind="ExternalInput")
    bias = nc.dram_tensor("bias", [P, OPP], mybir.dt.float32, kind="ExternalInput")
    y = nc.dram_tensor("y", [TOK, OPP], mybir.dt.float32, kind="ExternalOutput")

    # Batched-DMA views: one dma_start per x/y superblock (split across all
    # 16 SDMA engines), so the issuing engine isn't the bottleneck.
    xt_r = xt.rearrange("(k p) t -> p k t", p=P)        # [128, KT, TOK]
    y_r = y.rearrange("(s j p) c -> p s j c", p=P, j=TPS)  # [128, NSUP, TPS, OPP]

    with tile.TileContext(nc) as tc:
        with (
            tc.tile_pool(name="wpool", bufs=1) as wpool,
            tc.tile_pool(name="bpool", bufs=1) as bpool,
            tc.tile_pool(name="xpool", bufs=4) as xpool,
            tc.tile_pool(name="opool", bufs=2) as opool,
            tc.tile_pool(name="pspool", bufs=4, space="PSUM") as pspool,
        ):
            # First superblock + weights interleaved per-k so the first
            # accumulation group can start as soon as (w0, x0) land.
            w_sb = []
            x0_sb = xpool.tile([P, KT, TW], mm_dt, tag="x")
            for k in range(KT):
                nc.sync.dma_start(
                    out=x0_sb[:, k, :], in_=xt_r[:, k, 0:TW]
                )
                wtile = wpool.tile([P, OPP], mm_dt, tag=f"w{k}")
                nc.sync.dma_start(out=wtile[:], in_=wt[k * P : (k + 1) * P, :])
                w_sb.append(wtile)

            bias_sb = bpool.tile([P, OPP], mybir.dt.float32)
            nc.sync.dma_start(out=bias_sb[:], in_=bias[:])

            for s in range(NSUP):
                if s == 0:
                    x_sb = x0_sb
                else:
                    x_sb = xpool.tile([P, KT, TW], mm_dt, tag="x")
                    nc.sync.dma_start(
                        out=x_sb[:], in_=xt_r[:, :, s * TW : (s + 1) * TW]
                    )
                o_sb = opool.tile([P, TPS, OPP], mybir.dt.float32, tag="o")
                for j in range(TPS):
                    ps = pspool.tile([P, OPP], mybir.dt.float32)
                    for k in range(KT):
                        nc.tensor.matmul(
                            ps[:],
                            x_sb[:, k, j * P : (j + 1) * P],
                            w_sb[k][:],
                            start=(k == 0),
                            stop=(k == KT - 1),
                        )
                    nc.vector.tensor_add(o_sb[:, j, :], ps[:], bias_sb[:])
                # Stores go out on the scalar-engine HWDGE ring so they never
                # queue in front of the sync-ring loads.
                nc.scalar.dma_start(out=y_r[:, s, :, :], in_=o_sb[:])

    nc.compile()
    return nc, np_in


def _get_program(variant: str):
    if variant not in _programs:
        _programs[variant] = _build(variant)
    return _programs[variant]


def kernel(input_, idx_list, W, b, **_ignored):
    from concourse.bass_utils import run_bass_kernel_spmd

    input_ = np.asarray(input_)
    idx = np.asarray(idx_list).astype(np.int64)
    W = np.asarray(W, dtype=np.float32)
    b = np.asarray(b, dtype=np.float32)

    nc, np_in = _get_program(VARIANT)

    in_maps = []
    for e in range(E):
        xg = input_[idx[e]].reshape(TOK, D).astype(np.float32, copy=False)
        xtr = np.ascontiguousarray(xg.T).astype(np_in)
        wtr = np.ascontiguousarray(W[e].T).astype(np_in)
        bias_bc = np.ascontiguousarray(
            np.broadcast_to(b[e][None, :], (P, OPP))
        ).astype(np.float32)
        in_maps.append({"xt": xtr, "wt": wtr, "bias": bias_bc})

    res = run_bass_kernel_spmd(nc, in_maps, core_ids=list(range(E)))

    out = np.zeros((BS, S, E * OPP), dtype=input_.dtype)
    for e in range(E):
        ye = np.asarray(res.results[e]["y"], dtype=input_.dtype)
        out[idx[e], :, e * OPP : (e + 1) * OPP] = ye.reshape(BS // E, S, OPP)
    return out


# revision 8
# speedup vs baseline: 1.3489x; 1.2101x over previous
"""ColumnParallelLinearWithMoE Trainium2 kernel.

Expert-parallel: expert e -> NeuronCore e. Each core computes
    y_e [8192, 512] = x_e [8192, 1024] @ W_e.T [1024, 512] + b_e
where x_e = input_[idx_list[e]] flattened over (per, seq).

Routing gather/scatter and the x transpose (to put the contraction dim on
SBUF partitions) happen on the host; the device does the dense matmul.
"""

import sys

if "/opt/trn_rl_repo" not in sys.path:
    sys.path.insert(0, "/opt/trn_rl_repo")

import numpy as np

# Problem constants (hardcoded per harness contract).
E = 8
BS = 64
S = 1024
D = 1024
OPP = 512
P = 128
TOK = (BS // E) * S  # 8192 tokens per expert
KT = D // P          # 8 contraction tiles
TW = 512             # token-superblock width staged in SBUF
NSUP = TOK // TW
TPS = TW // P        # token tiles (of 128) per superblock

# Matmul dtype variant: "f32" (exact, slow), "f32r" (fast fp32 path),
# "bf16" (host-cast, fastest DMA).
VARIANT = "f32"

_programs: dict[str, tuple] = {}


def _build(variant: str):
    import concourse.bacc as bacc
    import concourse.tile as tile
    from concourse import mybir

    if variant == "f32":
        mm_dt = mybir.dt.float32
        np_in = np.float32
    elif variant == "f32r":
        mm_dt = mybir.dt.float32r
        np_in = np.float32
    elif variant == "bf16":
        import ml_dtypes

        mm_dt = mybir.dt.bfloat16
        np_in = ml_dtypes.bfloat16
    else:
        raise ValueError(variant)

    nc = bacc.Bacc(None, target_bir_lowering=False, debug=False)

    xt = nc.dram_tensor("xt", [D, TOK], mm_dt, kind="ExternalInput")
    wt = nc.dram_tensor("wt", [D, OPP], mm_dt, kind="ExternalInput")
    bias = nc.dram_tensor("bias", [P, OPP], mybir.dt.float32, kind="ExternalInput")
    y = nc.dram_tensor("y", [TOK, OPP], mybir.dt.float32, kind="ExternalOutput")

    # Batched-DMA views: one dma_start per x/y superblock (split across all
    # 16 SDMA engines), so the issuing engine isn't the bottleneck.
    xt_r = xt.rearrange("(k p) t -> p k t", p=P)        # [128, KT, TOK]
    wt_r = wt.rearrange("(k p) c -> p k c", p=P)        # [128, KT, OPP]
    y_r = y.rearrange("(s j p) c -> p s j c", p=P, j=TPS)  # [128, NSUP, TPS, OPP]

    with tile.TileContext(nc) as tc:
        with (
            tc.tile_pool(name="wpool", bufs=1) as wpool,
            tc.tile_pool(name="bpool", bufs=1) as bpool,
            tc.tile_pool(name="xpool", bufs=4) as xpool,
            tc.tile_pool(name="opool", bufs=2) as opool,
            tc.tile_pool(name="pspool", bufs=4, space="PSUM") as pspool,
        ):
            # Few, large DMAs: the HWDGE issue rate (~0.6us/dma_start) caps
            # early bandwidth if the first transfers are small.
            w_sb = wpool.tile([P, KT, OPP], mm_dt)
            nc.sync.dma_start(out=w_sb[:], in_=wt_r[:])

            bias_sb = bpool.tile([P, OPP], mybir.dt.float32)
            nc.sync.dma_start(out=bias_sb[:], in_=bias[:])

            for s in range(NSUP):
                x_sb = xpool.tile([P, KT, TW], mm_dt, tag="x")
                nc.sync.dma_start(
                    out=x_sb[:], in_=xt_r[:, :, s * TW : (s + 1) * TW]
                )
                o_sb = opool.tile([P, TPS, OPP], mybir.dt.float32, tag="o")
                for j in range(TPS):
                    ps = pspool.tile([P, OPP], mybir.dt.float32)
                    for k in range(KT):
                        nc.tensor.matmul(
                            ps[:],
                            x_sb[:, k, j * P : (j + 1) * P],
                            w_sb[:, k, :],
                            start=(k == 0),
                            stop=(k == KT - 1),
                        )
                    nc.vector.tensor_add(o_sb[:, j, :], ps[:], bias_sb[:])
                # Stores go out on the scalar-engine HWDGE ring so they never
                # queue in front of the sync-ring loads. The final super's
                # store is split per token-tile so the tail drains sooner.
                if s < NSUP - 1:
                    nc.scalar.dma_start(out=y_r[:, s, :, :], in_=o_sb[:])
                else:
                    for j in range(TPS):
                        nc.scalar.dma_start(
                            out=y_r[:, s, j, :], in_=o_sb[:, j, :]
                        )

    nc.compile()
    return nc, np_in


def _get_program(variant: str):
    if variant not in _programs:
        _programs[variant] = _build(variant)
    return _programs[variant]


def kernel(input_, idx_list, W, b, **_ignored):
    from concourse.bass_utils import run_bass_kernel_spmd

    input_ = np.asarray(input_)
    idx = np.asarray(idx_list).astype(np.int64)
    W = np.asarray(W, dtype=np.float32)
    b = np.asarray(b, dtype=np.float32)

    nc, np_in = _get_program(VARIANT)

    in_maps = []
    for e in range(E):
        xg = input_[idx[e]].reshape(TOK, D).astype(np.float32, copy=False)
        xtr = np.ascontiguousarray(xg.T).astype(np_in)
        wtr = np.ascontiguousarray(W[e].T).astype(np_in)
        bias_bc = np.ascontiguousarray(
            np.broadcast_to(b[e][None, :], (P, OPP))
        ).astype(np.float32)
        in_maps.append({"xt": xtr, "wt": wtr, "bias": bias_bc})

    res = run_bass_kernel_spmd(nc, in_maps, core_ids=list(range(E)))

    out = np.zeros((BS, S, E * OPP), dtype=input_.dtype)
    for e in range(E):
        ye = np.asarray(res.results[e]["y"], dtype=input_.dtype)
        out[idx[e], :, e * OPP : (e + 1) * OPP] = ye.reshape(BS // E, S, OPP)
    return out


# revision 9
# speedup vs baseline: 1.4460x; 1.0719x over previous
"""ColumnParallelLinearWithMoE Trainium2 kernel.

Expert-parallel: expert e -> NeuronCore e. Each core computes
    y_e [8192, 512] = x_e [8192, 1024] @ W_e.T [1024, 512] + b_e
where x_e = input_[idx_list[e]] flattened over (per, seq).

Routing gather/scatter and the x transpose (to put the contraction dim on
SBUF partitions) happen on the host; the device does the dense matmul.
"""

import sys

if "/opt/trn_rl_repo" not in sys.path:
    sys.path.insert(0, "/opt/trn_rl_repo")

import numpy as np

# Problem constants (hardcoded per harness contract).
E = 8
BS = 64
S = 1024
D = 1024
OPP = 512
P = 128
TOK = (BS // E) * S  # 8192 tokens per expert
KT = D // P          # 8 contraction tiles
TW = 512             # token-superblock width staged in SBUF
NSUP = TOK // TW
TPS = TW // P        # token tiles (of 128) per superblock

# Matmul dtype variant: "f32" (exact, slow), "f32r" (fast fp32 path),
# "bf16" (host-cast, fastest DMA).
VARIANT = "f32"

_programs: dict[str, tuple] = {}


def _build(variant: str):
    import concourse.bacc as bacc
    import concourse.tile as tile
    from concourse import mybir

    if variant == "f32":
        mm_dt = mybir.dt.float32
        np_in = np.float32
    elif variant == "f32r":
        mm_dt = mybir.dt.float32r
        np_in = np.float32
    elif variant == "bf16":
        import ml_dtypes

        mm_dt = mybir.dt.bfloat16
        np_in = ml_dtypes.bfloat16
    else:
        raise ValueError(variant)

    nc = bacc.Bacc(None, target_bir_lowering=False, debug=False)

    xt = nc.dram_tensor("xt", [D, TOK], mm_dt, kind="ExternalInput")
    wt = nc.dram_tensor("wt", [D, OPP], mm_dt, kind="ExternalInput")
    bias = nc.dram_tensor("bias", [P, OPP], mybir.dt.float32, kind="ExternalInput")
    y = nc.dram_tensor("y", [TOK, OPP], mybir.dt.float32, kind="ExternalOutput")

    # Batched-DMA views: one dma_start per x/y superblock (split across all
    # 16 SDMA engines), so the issuing engine isn't the bottleneck.
    xt_r = xt.rearrange("(k p) t -> p k t", p=P)        # [128, KT, TOK]
    wt_r = wt.rearrange("(k p) c -> p k c", p=P)        # [128, KT, OPP]
    y_r = y.rearrange("(s j p) c -> p s j c", p=P, j=TPS)  # [128, NSUP, TPS, OPP]

    with tile.TileContext(nc) as tc:
        with (
            tc.tile_pool(name="wpool", bufs=1) as wpool,
            tc.tile_pool(name="bpool", bufs=1) as bpool,
            tc.tile_pool(name="xpool", bufs=4) as xpool,
            tc.tile_pool(name="opool", bufs=2) as opool,
            tc.tile_pool(name="pspool", bufs=4, space="PSUM") as pspool,
        ):
            # PE prewarm: ~10 matmuls on a zeroed tile while the first loads
            # are in flight, so HAM un-throttles (1.2 -> 2.4 GHz) before the
            # first real matmul issues. Results are never read.
            warm_src = wpool.tile([P, OPP], mm_dt, tag="warm")
            nc.gpsimd.memset(warm_src[:], 0.0)
            warm_ps = pspool.tile([P, OPP], mybir.dt.float32, tag="warmps")
            for _ in range(10):
                nc.tensor.matmul(
                    warm_ps[:], warm_src[:, :P], warm_src[:], start=True, stop=True
                )

            # Few, large DMAs elsewhere (HWDGE issue rate ~0.6us/dma_start
            # caps early bandwidth), but x_s0 first and w split per-k so the
            # first accumulation group starts as soon as (x_s0, w0) land.
            x0_sb = xpool.tile([P, KT, TW], mm_dt, tag="x")
            nc.sync.dma_start(out=x0_sb[:], in_=xt_r[:, :, 0:TW])
            w_sb = wpool.tile([P, KT, OPP], mm_dt)
            for k in range(KT):
                nc.sync.dma_start(out=w_sb[:, k, :], in_=wt_r[:, k, :])

            bias_sb = bpool.tile([P, OPP], mybir.dt.float32)
            nc.sync.dma_start(out=bias_sb[:], in_=bias[:])

            for s in range(NSUP):
                if s == 0:
                    x_sb = x0_sb
                else:
                    x_sb = xpool.tile([P, KT, TW], mm_dt, tag="x")
                    nc.sync.dma_start(
                        out=x_sb[:], in_=xt_r[:, :, s * TW : (s + 1) * TW]
                    )
                o_sb = opool.tile([P, TPS, OPP], mybir.dt.float32, tag="o")
                for j in range(TPS):
                    ps = pspool.tile([P, OPP], mybir.dt.float32)
                    for k in range(KT):
                        nc.tensor.matmul(
                            ps[:],
                            x_sb[:, k, j * P : (j + 1) * P],
                            w_sb[:, k, :],
                            start=(k == 0),
                            stop=(k == KT - 1),
                        )
                    nc.vector.tensor_add(o_sb[:, j, :], ps[:], bias_sb[:])
                # Stores go out on the scalar-engine HWDGE ring so they never
                # queue in front of the sync-ring loads. The final super's
                # store is split per token-tile so the tail drains sooner.
                if s < NSUP - 1:
                    nc.scalar.dma_start(out=y_r[:, s, :, :], in_=o_sb[:])
                else:
                    for j in range(TPS):
                        nc.scalar.dma_start(
                            out=y_r[:, s, j, :], in_=o_sb[:, j, :]
                        )

    nc.compile()
    return nc, np_in


def _get_program(variant: str):
    if variant not in _programs:
        _programs[variant] = _build(variant)
    return _programs[variant]


def kernel(input_, idx_list, W, b, **_ignored):
    from concourse.bass_utils import run_bass_kernel_spmd

    input_ = np.asarray(input_)
    idx = np.asarray(idx_list).astype(np.int64)
    W = np.asarray(W, dtype=np.float32)
    b = np.asarray(b, dtype=np.float32)

    nc, np_in = _get_program(VARIANT)

    in_maps = []
    for e in range(E):
        xg = input_[idx[e]].reshape(TOK, D).astype(np.float32, copy=False)
        xtr = np.ascontiguousarray(xg.T).astype(np_in)
        wtr = np.ascontiguousarray(W[e].T).astype(np_in)
        bias_bc = np.ascontiguousarray(
            np.broadcast_to(b[e][None, :], (P, OPP))
        ).astype(np.float32)
        in_maps.append({"xt": xtr, "wt": wtr, "bias": bias_bc})

    res = run_bass_kernel_spmd(nc, in_maps, core_ids=list(range(E)))

    out = np.zeros((BS, S, E * OPP), dtype=input_.dtype)
    for e in range(E):
        ye = np.asarray(res.results[e]["y"], dtype=input_.dtype)
        out[idx[e], :, e * OPP : (e + 1) * OPP] = ye.reshape(BS // E, S, OPP)
    return out
